# revision 9
# baseline (speedup 1.0000x reference)
"""Chamfer-distance loss kernel for Trainium2 (8 NeuronCores, SPMD).

Math (masked ChamferDistanceLoss, see reference):
    pad = mx + (mx - mn) + 1 with mx/mn = max/min of (masked target max, centers max).
    mod_centers = centers + [pad];  mod_target = where(mask, target, pad)
    loss = mean_b [ sum_m min_n d2(mc_m, mt_n) + sum_n min_m d2(mt_n, mc_m) ]

Exact simplification used here (verified numerically against the reference):
  * pad >= 1 + max(values), all real values in [0,1), so
      - a padded (invalid) pixel's nearest mod_center is the pad center: contributes 0,
      - the pad center's nearest mod_target is a padded pixel: contributes 0,
      - a real pixel's nearest mod_center is never the pad center,
      - a real center's nearest mod_target is never a padded pixel.
    Hence both directions reduce to valid pixels x real 256 centers, and the
    global pad value (the only cross-shard coupling) cancels entirely.

Sharding: core k handles batch k//2, pixel half k%2 (38400 pixels, 256 centers).
Per-core device program (one Bass/Tile NEFF, SPMD on 8 cores):
  PE  : u[p, c] = t_p - c_c, exact fp32, via K=2 matmul  (lhsT = [t; 1], rhs = [1; -c])
  DVE : dir1 per-pixel min_c |u| (fp32, exact), via tensor_reduce(min, |.|)
  ACT : d2 = u^2 -> bf16
  DVE : dir2 acc = min(acc, d2 + BIG*(1-mask))  (bf16; dir2 term is ~1e-7 rel of total)
  epilogue: dir1 partial sum (mask-weighted) via ACT square + DVE + PE column-sum;
            dir2 per-center mins via PE transpose + DVE reduce.
Host: reshapes shards, then combines 8 x (1 scalar + 256 mins) partials.
"""

import numpy as np
from contextlib import ExitStack

B = 4
N_PIX = 240 * 320          # pixels per batch
HALF = N_PIX // 2          # 38400 pixels per core
C = 256                    # real centers per batch
PT = 128                   # partitions
TILES = HALF // PT         # 300 pixel tiles per core
BIG = 1.0e6                # added to masked-out pixels' d2 in dir2
ACC_INIT = 1.0e30

_CACHE = {}


def _build_nc():
    import concourse.bacc as bacc
    import concourse.tile as tile
    import concourse.mybir as mybir

    f32 = mybir.dt.float32
    bf16 = mybir.dt.bfloat16
    u8 = mybir.dt.uint8
    X = mybir.AxisListType.X
    OP = mybir.AluOpType
    AF = mybir.ActivationFunctionType

    nc = bacc.Bacc("TRN2", target_bir_lowering=False, debug=False)

    tb = nc.dram_tensor("tb", [2, HALF], f32, kind="ExternalInput")
    coef = nc.dram_tensor("coef", [2, C], f32, kind="ExternalInput")
    mask8 = nc.dram_tensor("mask8", [PT, TILES], u8, kind="ExternalInput")
    ident_in = nc.dram_tensor("ident", [PT, PT], f32, kind="ExternalInput")
    out_s1 = nc.dram_tensor("out_s1", [1, 1], f32, kind="ExternalOutput")
    out_m2 = nc.dram_tensor("out_m2", [PT, 2], f32, kind="ExternalOutput")

    with tile.TileContext(nc) as tc, ExitStack() as ctx:
        singles = ctx.enter_context(tc.tile_pool(name="singles", bufs=1))
        psum = ctx.enter_context(tc.tile_pool(name="psum", bufs=4, space="PSUM"))
        psum_ep = ctx.enter_context(tc.tile_pool(name="psum_ep", bufs=1, space="PSUM"))
        d2p = ctx.enter_context(tc.tile_pool(name="d2p", bufs=4))

        tb_s = singles.tile([2, HALF], f32)
        nc.sync.dma_start(out=tb_s, in_=tb[:, :])
        coef_s = singles.tile([2, C], f32)
        nc.sync.dma_start(out=coef_s, in_=coef[:, :])
        m8 = singles.tile([PT, TILES], u8)
        nc.sync.dma_start(out=m8, in_=mask8[:, :])

        maskf = singles.tile([PT, TILES], f32)
        nc.vector.tensor_copy(out=maskf, in_=m8)
        # bigm = BIG * (1 - mask): 0 for valid pixels, BIG for masked-out ones
        bigm = singles.tile([PT, TILES], f32)
        nc.vector.tensor_scalar(
            out=bigm, in0=maskf, scalar1=-BIG, scalar2=BIG, op0=OP.mult, op1=OP.add
        )

        d1min = singles.tile([PT, TILES], f32)
        acc = singles.tile([PT, C], bf16)
        nc.vector.memset(acc, ACC_INIT)
        ident = singles.tile([PT, PT], f32)
        nc.sync.dma_start(out=ident, in_=ident_in[:, :])

        for j in range(TILES):
            u = psum.tile([PT, C], f32)
            nc.tensor.matmul(
                u, lhsT=tb_s[:, j * PT:(j + 1) * PT], rhs=coef_s,
                start=True, stop=True,
            )
            # dir1: exact per-pixel min over centers of |t - c| (square later)
            nc.vector.tensor_reduce(
                out=d1min[:, j:j + 1], in_=u, axis=X, op=OP.min,
                apply_absolute_value=True,
            )
            # d2 in bf16 for dir2 only
            d2 = d2p.tile([PT, C], bf16)
            nc.scalar.activation(out=d2, in_=u, func=AF.Square)
            # dir2: acc = min(acc, d2 + bigm_col)
            nc.vector.scalar_tensor_tensor(
                out=acc, in0=d2, scalar=bigm[:, j:j + 1], in1=acc,
                op0=OP.add, op1=OP.min,
            )

        # ---- epilogue ----
        # dir1 partial: sum over valid pixels of (min |t-c|)^2
        d1sq = singles.tile([PT, TILES], f32)
        nc.scalar.activation(out=d1sq, in_=d1min, func=AF.Square)
        d1m = singles.tile([PT, TILES], f32)
        nc.vector.tensor_tensor(out=d1m, in0=d1sq, in1=maskf, op=OP.mult)
        rowsum = singles.tile([PT, 1], f32)
        nc.vector.tensor_reduce(out=rowsum, in_=d1m, axis=X, op=OP.add)
        ones_s = singles.tile([PT, 1], f32)
        nc.vector.memset(ones_s, 1.0)
        s1p = psum_ep.tile([1, 1], f32)
        nc.tensor.matmul(s1p, lhsT=rowsum, rhs=ones_s, start=True, stop=True)
        s1s = singles.tile([1, 1], f32)
        nc.vector.tensor_copy(out=s1s, in_=s1p)
        nc.sync.dma_start(out=out_s1[:, :], in_=s1s)

        # dir2: per-center min over this core's pixels
        accf = singles.tile([PT, C], f32)
        nc.vector.tensor_copy(out=accf, in_=acc)
        m2 = singles.tile([PT, 2], f32)
        for g in range(2):
            trp = psum_ep.tile([PT, PT], f32)
            nc.tensor.transpose(trp, accf[:, g * PT:(g + 1) * PT], ident)
            nc.vector.tensor_reduce(out=m2[:, g:g + 1], in_=trp, axis=X, op=OP.min)
        nc.sync.dma_start(out=out_m2[:, :], in_=m2)

    nc.finalize()
    return nc


def _get_nc():
    if "nc" not in _CACHE:
        _CACHE["nc"] = _build_nc()
    return _CACHE["nc"]


def _in_maps(target, bin_centers, mask):
    target = np.asarray(target, dtype=np.float32)
    bin_centers = np.asarray(bin_centers, dtype=np.float32)
    mask = np.asarray(mask)
    ones_row = np.ones(HALF, dtype=np.float32)
    ones_c = np.ones(C, dtype=np.float32)
    maps = []
    for k in range(8):
        b, h = divmod(k, 2)
        t_half = target[b].reshape(-1)[h * HALF:(h + 1) * HALF]
        m_half = mask[b].reshape(-1)[h * HALF:(h + 1) * HALF]
        maps.append({
            "tb": np.ascontiguousarray(np.stack([t_half, ones_row])),
            "coef": np.ascontiguousarray(np.stack([ones_c, -bin_centers[b]])),
            # mask8[p, j] corresponds to pixel j*128 + p of this core's shard
            "mask8": np.ascontiguousarray(
                m_half.astype(np.uint8).reshape(TILES, PT).T
            ),
            "ident": np.eye(PT, dtype=np.float32),
        })
    return maps


def _combine(results):
    s1 = np.array([results[k]["out_s1"][0, 0] for k in range(8)], dtype=np.float32)
    m2 = np.stack([
        results[k]["out_m2"].T.reshape(-1).astype(np.float32) for k in range(8)
    ])  # (8, 256); row k = per-center min over core k's pixels
    total = np.float32(0.0)
    for b in range(B):
        d1 = s1[2 * b] + s1[2 * b + 1]
        d2 = np.minimum(m2[2 * b], m2[2 * b + 1]).sum(dtype=np.float32)
        total += d1 + d2
    return np.float32(total / B)


def kernel(target, bin_centers, mask, _trace=False, _trace_kwargs=None):
    from concourse.bass_utils import run_bass_kernel_spmd

    nc = _get_nc()
    maps = _in_maps(target, bin_centers, mask)
    res = run_bass_kernel_spmd(
        nc, maps, core_ids=list(range(8)), trace=_trace,
        **(_trace_kwargs or {}),
    )
    out = _combine(res.results)
    if _trace:
        return out, res
    return out


# revision 22
# speedup vs baseline: 1.5527x; 1.5527x over previous
"""Chamfer-distance loss kernel for Trainium2 (8 NeuronCores, SPMD).

Math (masked ChamferDistanceLoss, see reference):
    pad = mx + (mx - mn) + 1 with mx/mn = max/min of (masked target max, centers max).
    mod_centers = centers + [pad];  mod_target = where(mask, target, pad)
    loss = mean_b [ sum_m min_n d2(mc_m, mt_n) + sum_n min_m d2(mt_n, mc_m) ]

Exact simplification used here (verified numerically against the reference):
  * pad >= 1 + max(values), all real values in [0,1), so
      - a padded (invalid) pixel's nearest mod_center is the pad center: contributes 0,
      - the pad center's nearest mod_target is a padded pixel: contributes 0,
      - a real pixel's nearest mod_center is never the pad center,
      - a real center's nearest mod_target is never a padded pixel.
    Hence both directions reduce to valid pixels x real 256 centers, and the
    global pad value (the only cross-shard coupling) cancels entirely.

Sharding: core k handles batch k//2, pixel half k%2 (38400 pixels, 256 centers).
Per-core device program (one Bass/Tile NEFF, SPMD on 8 cores):
  PE  : u[p, c] = t_p - c_c, exact fp32, via K=2 matmul  (lhsT = [t; 1], rhs = [1; -c])
  DVE : dir1 per-pixel min_c |u| (fp32, exact), via tensor_reduce(min, |.|)
  ACT : d2 = u^2 -> bf16
  DVE : dir2 acc = min(acc, d2 + BIG*(1-mask))  (bf16; dir2 term is ~1e-7 rel of total)
  epilogue: dir1 partial sum (mask-weighted) via ACT square + DVE + PE column-sum;
            dir2 per-center mins via PE transpose + DVE reduce.
Host: reshapes shards, then combines 8 x (1 scalar + 256 mins) partials.
"""

import numpy as np
from contextlib import ExitStack

B = 4
N_PIX = 240 * 320          # pixels per batch
HALF = N_PIX // 2          # 38400 pixels per core
C = 256                    # real centers per batch
PT = 128                   # partitions
TILES = HALF // PT         # 300 pixel tiles per core
BIG = 1.0e6                # added to masked-out pixels' d2 in dir2
ACC_INIT = 1.0e30

_CACHE = {}


def _register_dve_op(name, spec, subdim=False):
    """Register a custom DVE op at runtime (the repo registry is read-only)."""
    import concourse.dve_ops as dve_ops
    from concourse.dve_spec import lower, _has_src1
    from concourse.dve_uop import DveOpSpec

    for op in dve_ops.OPS:
        if op.name == name:
            return op
    row = dve_ops._CUSTOM_DVE_ROW_BASE + len(dve_ops.OPS)
    assert row < 0x20
    shas = {}
    for ver in ("v3",):
        uops = lower(spec, ver=ver)
        tmp = DveOpSpec(name=name, opcode=row, uops=uops, rd1_en=_has_src1(spec))
        shas[ver] = tmp.sha(ver)
    op = dve_ops.DveOp(name, spec, subdim=subdim, uops_sha=shas)
    dve_ops.OPS.append(op)
    dve_ops._SUB_OPCODE_FOR_NAME[name] = row
    dve_ops.CUSTOM_DVE_SPECS[name] = spec
    return op


def _chamfer_d2_op():
    """out[p,k] = (in0[p,k] + s0[p])^2 ; accum_out[p] = min(s1, min_k out)"""
    from concourse.dve_spec import Spec, Src0, C0, C1, sq, minn

    def _ref(in0, in1, s0, s1, imm2):
        b = ((in0.astype(np.float32) + s0) ** 2).astype(np.float32)
        a = np.minimum(
            np.asarray(s1, np.float32),
            b.reshape(b.shape[0], -1).min(axis=-1, keepdims=True),
        )
        return b, a

    return _register_dve_op(
        "CHAMFER_D2_ANT",
        Spec(body=sq(Src0 + C0), accum=minn, accum_init=C1, reference=_ref),
    )


def _build_nc():
    import concourse.bacc as bacc
    import concourse.tile as tile
    import concourse.mybir as mybir

    f32 = mybir.dt.float32
    bf16 = mybir.dt.bfloat16
    u8 = mybir.dt.uint8
    X = mybir.AxisListType.X
    OP = mybir.AluOpType
    AF = mybir.ActivationFunctionType

    nc = bacc.Bacc("TRN2", target_bir_lowering=False, debug=False)

    tpix = nc.dram_tensor("tpix", [PT, TILES], f32, kind="ExternalInput")
    cb = nc.dram_tensor("cb", [PT, C], f32, kind="ExternalInput")
    mask8 = nc.dram_tensor("mask8", [PT, TILES], u8, kind="ExternalInput")
    ident_in = nc.dram_tensor("ident", [PT, PT], f32, kind="ExternalInput")
    out_s1 = nc.dram_tensor("out_s1", [1, 1], f32, kind="ExternalOutput")
    out_m2 = nc.dram_tensor("out_m2", [PT, 2], f32, kind="ExternalOutput")

    with tile.TileContext(nc) as tc, ExitStack() as ctx:
        singles = ctx.enter_context(tc.tile_pool(name="singles", bufs=1))
        psum_ep = ctx.enter_context(tc.tile_pool(name="psum_ep", bufs=1, space="PSUM"))
        d2p = ctx.enter_context(tc.tile_pool(name="d2p", bufs=6))

        t_s = singles.tile([PT, TILES], f32)
        nc.sync.dma_start(out=t_s, in_=tpix[:, :])
        cb_s = singles.tile([PT, C], f32)
        nc.sync.dma_start(out=cb_s, in_=cb[:, :])
        m8 = singles.tile([PT, TILES], u8)
        nc.sync.dma_start(out=m8, in_=mask8[:, :])

        maskf = singles.tile([PT, TILES], f32)
        nc.vector.tensor_copy(out=maskf, in_=m8)
        # negt = -(mask ? t : 2.0): 2.0 is farther from every center than any
        # real pixel, so masked-out pixels never win a dir2 min (and dir1
        # drops them via the mask weight).
        negt_all = singles.tile([PT, TILES], f32)
        nc.vector.tensor_scalar(
            out=negt_all, in0=t_s, scalar1=-1.0, scalar2=None, op0=OP.mult
        )
        negt = singles.tile([PT, TILES], f32)
        nc.vector.memset(negt, -2.0)
        nc.vector.copy_predicated(out=negt, mask=m8, data=negt_all)

        d1min = singles.tile([PT, TILES], f32)
        QUAD = 4
        acc4 = singles.tile([PT, QUAD, C], bf16)
        nc.vector.memset(acc4, ACC_INIT)
        ident = singles.tile([PT, PT], f32)
        nc.sync.dma_start(out=ident, in_=ident_in[:, :])

        ch_op = _chamfer_d2_op()
        for jq in range(TILES // QUAD):
            # CHAMFER_D2: d2m (bf16) = (c - t_j)^2, accum = exact fp32 min -> dir1
            quad = d2p.tile([PT, QUAD, C], bf16, tag="quad")
            for q in range(QUAD):
                j = jq * QUAD + q
                nc.vector._custom_dve(
                    ch_op,
                    out=quad[:, q, :],
                    in0=cb_s,
                    s0=negt[:, j:j + 1],
                    s1=ACC_INIT,
                    accum_out=d1min[:, j:j + 1],
                )
            # dir2: one batched bf16 min-accumulate per 4 tiles
            nc.vector.tensor_tensor(out=acc4, in0=acc4, in1=quad, op=OP.min)

        # ---- epilogue ----
        # dir1 partial: sum over valid pixels of min_c (t-c)^2
        d1m = singles.tile([PT, TILES], f32)
        nc.vector.tensor_tensor(out=d1m, in0=d1min, in1=maskf, op=OP.mult)
        rowsum = singles.tile([PT, 1], f32)
        nc.vector.tensor_reduce(out=rowsum, in_=d1m, axis=X, op=OP.add)
        ones_s = singles.tile([PT, 1], f32)
        nc.vector.memset(ones_s, 1.0)
        s1p = psum_ep.tile([1, 1], f32)
        nc.tensor.matmul(s1p, lhsT=rowsum, rhs=ones_s, start=True, stop=True)
        s1s = singles.tile([1, 1], f32)
        nc.vector.tensor_copy(out=s1s, in_=s1p)
        nc.sync.dma_start(out=out_s1[:, :], in_=s1s)

        # dir2: merge acc quad slots, then per-center min over this core's pixels
        nc.vector.tensor_tensor(
            out=acc4[:, 0:2, :], in0=acc4[:, 0:2, :], in1=acc4[:, 2:4, :], op=OP.min
        )
        nc.vector.tensor_tensor(
            out=acc4[:, 0, :], in0=acc4[:, 0, :], in1=acc4[:, 1, :], op=OP.min
        )
        accf = singles.tile([PT, C], f32)
        nc.vector.tensor_copy(out=accf, in_=acc4[:, 0, :])
        m2 = singles.tile([PT, 2], f32)
        for g in range(2):
            trp = psum_ep.tile([PT, PT], f32)
            nc.tensor.transpose(trp, accf[:, g * PT:(g + 1) * PT], ident)
            nc.vector.tensor_reduce(out=m2[:, g:g + 1], in_=trp, axis=X, op=OP.min)
        nc.sync.dma_start(out=out_m2[:, :], in_=m2)

    nc.finalize()
    return nc


def _get_nc():
    if "nc" not in _CACHE:
        _CACHE["nc"] = _build_nc()
    return _CACHE["nc"]


def _in_maps(target, bin_centers, mask):
    target = np.asarray(target, dtype=np.float32)
    bin_centers = np.asarray(bin_centers, dtype=np.float32)
    mask = np.asarray(mask)
    ident = np.eye(PT, dtype=np.float32)
    maps = []
    for k in range(8):
        b, h = divmod(k, 2)
        t_half = target[b].reshape(-1)[h * HALF:(h + 1) * HALF]
        m_half = mask[b].reshape(-1)[h * HALF:(h + 1) * HALF]
        maps.append({
            # [p, j] corresponds to pixel j*128 + p of this core's shard
            "tpix": np.ascontiguousarray(t_half.reshape(TILES, PT).T),
            "cb": np.ascontiguousarray(
                np.broadcast_to(bin_centers[b], (PT, C))
            ),
            "mask8": np.ascontiguousarray(
                m_half.astype(np.uint8).reshape(TILES, PT).T
            ),
            "ident": ident,
        })
    return maps


def _combine(results):
    s1 = np.array([results[k]["out_s1"][0, 0] for k in range(8)], dtype=np.float32)
    m2 = np.stack([
        results[k]["out_m2"].T.reshape(-1).astype(np.float32) for k in range(8)
    ])  # (8, 256); row k = per-center min over core k's pixels
    total = np.float32(0.0)
    for b in range(B):
        d1 = s1[2 * b] + s1[2 * b + 1]
        d2 = np.minimum(m2[2 * b], m2[2 * b + 1]).sum(dtype=np.float32)
        total += d1 + d2
    return np.float32(total / B)


def kernel(target, bin_centers, mask, _trace=False, _trace_kwargs=None):
    from concourse.bass_utils import run_bass_kernel_spmd

    nc = _get_nc()
    maps = _in_maps(target, bin_centers, mask)
    res = run_bass_kernel_spmd(
        nc, maps, core_ids=list(range(8)), trace=_trace,
        **(_trace_kwargs or {}),
    )
    out = _combine(res.results)
    if _trace:
        return out, res
    return out


# revision 24
# speedup vs baseline: 1.8365x; 1.1828x over previous
"""Chamfer-distance loss kernel for Trainium2 (8 NeuronCores, SPMD).

Math (masked ChamferDistanceLoss, see reference):
    pad = mx + (mx - mn) + 1 with mx/mn = max/min of (masked target max, centers max).
    mod_centers = centers + [pad];  mod_target = where(mask, target, pad)
    loss = mean_b [ sum_m min_n d2(mc_m, mt_n) + sum_n min_m d2(mt_n, mc_m) ]

Exact simplification used here (verified numerically against the reference):
  * pad >= 1 + max(values), all real values in [0,1), so
      - a padded (invalid) pixel's nearest mod_center is the pad center: contributes 0,
      - the pad center's nearest mod_target is a padded pixel: contributes 0,
      - a real pixel's nearest mod_center is never the pad center,
      - a real center's nearest mod_target is never a padded pixel.
    Hence both directions reduce to valid pixels x real 256 centers, and the
    global pad value (the only cross-shard coupling) cancels entirely.

Sharding: core k handles batch k//2, pixel half k%2 (38400 pixels, 256 centers).
Per-core device program (one Bass/Tile NEFF, SPMD on 8 cores):
  PE  : u[p, c] = t_p - c_c, exact fp32, via K=2 matmul  (lhsT = [t; 1], rhs = [1; -c])
  DVE : dir1 per-pixel min_c |u| (fp32, exact), via tensor_reduce(min, |.|)
  ACT : d2 = u^2 -> bf16
  DVE : dir2 acc = min(acc, d2 + BIG*(1-mask))  (bf16; dir2 term is ~1e-7 rel of total)
  epilogue: dir1 partial sum (mask-weighted) via ACT square + DVE + PE column-sum;
            dir2 per-center mins via PE transpose + DVE reduce.
Host: reshapes shards, then combines 8 x (1 scalar + 256 mins) partials.
"""

import numpy as np
from contextlib import ExitStack

B = 4
N_PIX = 240 * 320          # pixels per batch
HALF = N_PIX // 2          # 38400 pixels per core
C = 256                    # real centers per batch
PT = 128                   # partitions
TILES = HALF // PT         # 300 pixel tiles per core
BIG = 1.0e6                # added to masked-out pixels' d2 in dir2
ACC_INIT = 1.0e30

_CACHE = {}


def _register_dve_op(name, spec, subdim=False):
    """Register a custom DVE op at runtime (the repo registry is read-only)."""
    import concourse.dve_ops as dve_ops
    from concourse.dve_spec import lower, _has_src1
    from concourse.dve_uop import DveOpSpec

    for op in dve_ops.OPS:
        if op.name == name:
            return op
    row = dve_ops._CUSTOM_DVE_ROW_BASE + len(dve_ops.OPS)
    assert row < 0x20
    shas = {}
    for ver in ("v3",):
        uops = lower(spec, ver=ver)
        tmp = DveOpSpec(name=name, opcode=row, uops=uops, rd1_en=_has_src1(spec))
        shas[ver] = tmp.sha(ver)
    op = dve_ops.DveOp(name, spec, subdim=subdim, uops_sha=shas)
    dve_ops.OPS.append(op)
    dve_ops._SUB_OPCODE_FOR_NAME[name] = row
    dve_ops.CUSTOM_DVE_SPECS[name] = spec
    return op


def _chamfer_d2_op():
    """out[p,k] = (in0[p,k] + s0[p])^2 ; accum_out[p] = min(s1, min_k out)"""
    from concourse.dve_spec import Spec, Src0, C0, C1, sq, minn

    def _ref(in0, in1, s0, s1, imm2):
        b = ((in0.astype(np.float32) + s0) ** 2).astype(np.float32)
        a = np.minimum(
            np.asarray(s1, np.float32),
            b.reshape(b.shape[0], -1).min(axis=-1, keepdims=True),
        )
        return b, a

    return _register_dve_op(
        "CHAMFER_D2_ANT",
        Spec(body=sq(Src0 + C0), accum=minn, accum_init=C1, reference=_ref),
    )


def _chamfer_fold_op():
    """out[p,k] = min((in0[p,k]+s0[p])^2, (in1[p,k]+s0[p])^2);
    accum_out[p] = min(s1, min_k out) — dir1 min over both center halves,
    scanning 2 centers per cycle."""
    from concourse.dve_spec import Spec, Src0, Src1, C0, C1, sq, minn

    def _ref(in0, in1, s0, s1, imm2):
        b = np.minimum(
            (in0.astype(np.float32) + s0) ** 2,
            (in1.astype(np.float32) + s0) ** 2,
        ).astype(np.float32)
        a = np.minimum(
            np.asarray(s1, np.float32),
            b.reshape(b.shape[0], -1).min(axis=-1, keepdims=True),
        )
        return b, a

    return _register_dve_op(
        "CHAMFER_FOLD_ANT",
        Spec(
            body=minn(sq(Src0 + C0), sq(Src1 + C0)),
            accum=minn,
            accum_init=C1,
            reference=_ref,
        ),
    )


def _build_nc():
    import concourse.bacc as bacc
    import concourse.tile as tile
    import concourse.mybir as mybir

    f32 = mybir.dt.float32
    bf16 = mybir.dt.bfloat16
    u8 = mybir.dt.uint8
    X = mybir.AxisListType.X
    OP = mybir.AluOpType
    AF = mybir.ActivationFunctionType

    nc = bacc.Bacc("TRN2", target_bir_lowering=False, debug=False)

    tpix = nc.dram_tensor("tpix", [PT, TILES], f32, kind="ExternalInput")
    cb = nc.dram_tensor("cb", [PT, C], f32, kind="ExternalInput")
    mask8 = nc.dram_tensor("mask8", [PT, TILES], u8, kind="ExternalInput")
    ident_in = nc.dram_tensor("ident", [PT, PT], f32, kind="ExternalInput")
    out_s1 = nc.dram_tensor("out_s1", [1, 1], f32, kind="ExternalOutput")
    out_m2 = nc.dram_tensor("out_m2", [PT, 2], f32, kind="ExternalOutput")

    with tile.TileContext(nc) as tc, ExitStack() as ctx:
        singles = ctx.enter_context(tc.tile_pool(name="singles", bufs=1))
        psum_ep = ctx.enter_context(tc.tile_pool(name="psum_ep", bufs=1, space="PSUM"))
        d2p = ctx.enter_context(tc.tile_pool(name="d2p", bufs=6))

        t_s = singles.tile([PT, TILES], f32)
        nc.sync.dma_start(out=t_s, in_=tpix[:, :])
        cb_s = singles.tile([PT, C], f32)
        nc.sync.dma_start(out=cb_s, in_=cb[:, :])
        m8 = singles.tile([PT, TILES], u8)
        nc.sync.dma_start(out=m8, in_=mask8[:, :])

        maskf = singles.tile([PT, TILES], f32)
        nc.vector.tensor_copy(out=maskf, in_=m8)
        # negt = -(mask ? t : 2.0): 2.0 is farther from every center than any
        # real pixel, so masked-out pixels never win a dir2 min (and dir1
        # drops them via the mask weight).
        negt_all = singles.tile([PT, TILES], f32)
        nc.vector.tensor_scalar(
            out=negt_all, in0=t_s, scalar1=-1.0, scalar2=None, op0=OP.mult
        )
        negt = singles.tile([PT, TILES], f32)
        nc.vector.memset(negt, -2.0)
        nc.vector.copy_predicated(out=negt, mask=m8, data=negt_all)

        d1min = singles.tile([PT, TILES], f32)
        QUAD = 4
        acc4 = singles.tile([PT, QUAD, C], bf16)
        nc.vector.memset(acc4, ACC_INIT)
        ident = singles.tile([PT, PT], f32)
        nc.sync.dma_start(out=ident, in_=ident_in[:, :])

        ch_op = _chamfer_d2_op()
        fold_op = _chamfer_fold_op()
        # Per 4-tile quad: tile 0 runs path-P (CHAMFER_D2 on DVE produces both
        # d2m and dir1); tiles 1-3 run path-F (dir1 via the 2x-fast FOLD op on
        # DVE, d2m produced by the otherwise-idle Scalar engine).
        N_PATH_P = 1
        for jq in range(TILES // QUAD):
            quad = d2p.tile([PT, QUAD, C], bf16, tag="quad")
            for q in range(QUAD):
                j = jq * QUAD + q
                if q < N_PATH_P:
                    nc.vector._custom_dve(
                        ch_op,
                        out=quad[:, q, :],
                        in0=cb_s,
                        s0=negt[:, j:j + 1],
                        s1=ACC_INIT,
                        accum_out=d1min[:, j:j + 1],
                    )
                else:
                    fscr = d2p.tile([PT, C // 2], bf16, tag="fscr")
                    nc.vector._custom_dve(
                        fold_op,
                        out=fscr,
                        in0=cb_s[:, 0:C // 2],
                        in1=cb_s[:, C // 2:C],
                        s0=negt[:, j:j + 1],
                        s1=ACC_INIT,
                        accum_out=d1min[:, j:j + 1],
                    )
                    nc.scalar.activation(
                        out=quad[:, q, :], in_=cb_s, func=AF.Square,
                        bias=negt[:, j:j + 1],
                    )
            # dir2: one batched bf16 min-accumulate per 4 tiles
            nc.vector.tensor_tensor(out=acc4, in0=acc4, in1=quad, op=OP.min)

        # ---- epilogue ----
        # dir1 partial: sum over valid pixels of min_c (t-c)^2
        d1m = singles.tile([PT, TILES], f32)
        nc.vector.tensor_tensor(out=d1m, in0=d1min, in1=maskf, op=OP.mult)
        rowsum = singles.tile([PT, 1], f32)
        nc.vector.tensor_reduce(out=rowsum, in_=d1m, axis=X, op=OP.add)
        ones_s = singles.tile([PT, 1], f32)
        nc.vector.memset(ones_s, 1.0)
        s1p = psum_ep.tile([1, 1], f32)
        nc.tensor.matmul(s1p, lhsT=rowsum, rhs=ones_s, start=True, stop=True)
        s1s = singles.tile([1, 1], f32)
        nc.vector.tensor_copy(out=s1s, in_=s1p)
        nc.sync.dma_start(out=out_s1[:, :], in_=s1s)

        # dir2: merge acc quad slots, then per-center min over this core's pixels
        nc.vector.tensor_tensor(
            out=acc4[:, 0:2, :], in0=acc4[:, 0:2, :], in1=acc4[:, 2:4, :], op=OP.min
        )
        nc.vector.tensor_tensor(
            out=acc4[:, 0, :], in0=acc4[:, 0, :], in1=acc4[:, 1, :], op=OP.min
        )
        accf = singles.tile([PT, C], f32)
        nc.vector.tensor_copy(out=accf, in_=acc4[:, 0, :])
        m2 = singles.tile([PT, 2], f32)
        for g in range(2):
            trp = psum_ep.tile([PT, PT], f32)
            nc.tensor.transpose(trp, accf[:, g * PT:(g + 1) * PT], ident)
            nc.vector.tensor_reduce(out=m2[:, g:g + 1], in_=trp, axis=X, op=OP.min)
        nc.sync.dma_start(out=out_m2[:, :], in_=m2)

    nc.finalize()
    return nc


def _get_nc():
    if "nc" not in _CACHE:
        _CACHE["nc"] = _build_nc()
    return _CACHE["nc"]


def _in_maps(target, bin_centers, mask):
    target = np.asarray(target, dtype=np.float32)
    bin_centers = np.asarray(bin_centers, dtype=np.float32)
    mask = np.asarray(mask)
    ident = np.eye(PT, dtype=np.float32)
    maps = []
    for k in range(8):
        b, h = divmod(k, 2)
        t_half = target[b].reshape(-1)[h * HALF:(h + 1) * HALF]
        m_half = mask[b].reshape(-1)[h * HALF:(h + 1) * HALF]
        maps.append({
            # [p, j] corresponds to pixel j*128 + p of this core's shard
            "tpix": np.ascontiguousarray(t_half.reshape(TILES, PT).T),
            "cb": np.ascontiguousarray(
                np.broadcast_to(bin_centers[b], (PT, C))
            ),
            "mask8": np.ascontiguousarray(
                m_half.astype(np.uint8).reshape(TILES, PT).T
            ),
            "ident": ident,
        })
    return maps


def _combine(results):
    s1 = np.array([results[k]["out_s1"][0, 0] for k in range(8)], dtype=np.float32)
    m2 = np.stack([
        results[k]["out_m2"].T.reshape(-1).astype(np.float32) for k in range(8)
    ])  # (8, 256); row k = per-center min over core k's pixels
    total = np.float32(0.0)
    for b in range(B):
        d1 = s1[2 * b] + s1[2 * b + 1]
        d2 = np.minimum(m2[2 * b], m2[2 * b + 1]).sum(dtype=np.float32)
        total += d1 + d2
    return np.float32(total / B)


def kernel(target, bin_centers, mask, _trace=False, _trace_kwargs=None):
    from concourse.bass_utils import run_bass_kernel_spmd

    nc = _get_nc()
    maps = _in_maps(target, bin_centers, mask)
    res = run_bass_kernel_spmd(
        nc, maps, core_ids=list(range(8)), trace=_trace,
        **(_trace_kwargs or {}),
    )
    out = _combine(res.results)
    if _trace:
        return out, res
    return out
